# revision 13
# baseline (speedup 1.0000x reference)
"""MoE expert-routing kernel for Trainium2 (8 NeuronCores, expert-parallel).

Problem: out[t] = x[t] @ weight[index[t]] + bias[index[t]]
  x: (32768, 512) f32, index: (32768,) int, weight: (8, 512, 512) f32,
  bias: (8, 512) f32.

Strategy (expert-parallel, host-side dispatch):
  Core e owns expert e. The host gathers the tokens routed to expert e
  into a fixed-capacity, transposed buffer xt_e[512, CAP] (padded with
  zeros), and core e computes y_e = x_e @ W_e + b_e as a single dense
  GEMM. Results are scattered back to token order on the host. Tokens
  beyond CAP (never happens for the benchmark distribution: binomial
  max ~4205 << CAP) fall back to a host matmul for correctness.

Device kernel (per core): y[CAP, 512] = xt.T @ w + b
  - xt arrives pre-transposed [512(k), CAP(t)] so the K-contraction dim
    lands on SBUF partitions with no on-device transposes.
  - Token slabs of 512 stream through SBUF; per 128-token tile, 4
    accumulating matmuls (K=128 chunks) into one PSUM bank; DVE adds the
    (pre-replicated) bias while moving PSUM->SBUF; slab-sized output DMA.
"""

import os

import numpy as np

N_EXPERTS = 8
D_IN = 512
D_OUT = 512
N_TOKENS = 32768
CAP = 4352  # per-expert token capacity: 34*128, binomial(32768,1/8) max +4sigma
TOK_SLAB = 1024
KC = D_IN // 128  # 4 contraction chunks

# Matmul operand dtypes (x, w):
#   "float32"  = full fp32 (4 cyc/row on PE, ~1.6e-7 rel err)
#   "float32r" = PE fast-fp32 (tf32-like, ~1.4e-4 rel err)
#   "bf16x"    = x in bf16, w in float32r (~? rel err, halves x DMA)
#   "bfloat16" = both bf16 (~2e-3 rel err, min DMA)
MM_DTYPE = os.environ.get("KERNEL_MM_DTYPE", "float32r_o16")
# mode -> (x dtype, w dtype, y dtype)
_DT_MAP = {
    "float32": ("float32", "float32", "float32"),
    "float32r": ("float32r", "float32r", "float32"),
    "float32r_o16": ("float32r", "float32r", "float16"),
    "bf16x": ("bfloat16", "float32r", "float32"),
    "bfloat16": ("bfloat16", "bfloat16", "float32"),
    "float16": ("float16", "float16", "float32"),
    "float16_o16": ("float16", "float16", "float16"),
}

_cache = {}


def _build(mm_dtype_name):
    import concourse.bacc as bacc
    import concourse.mybir as mybir
    import concourse.tile as tile

    x_dt_name, w_dt_name, y_dt_name = _DT_MAP[mm_dtype_name]
    dt_x = getattr(mybir.dt, x_dt_name)
    dt_w = getattr(mybir.dt, w_dt_name)
    dt_y = getattr(mybir.dt, y_dt_name)
    f32 = mybir.dt.float32

    nc = bacc.Bacc("TRN2", target_bir_lowering=False, debug=False, num_devices=N_EXPERTS)
    xt = nc.dram_tensor("xt", (D_IN, CAP), dt_x, kind="ExternalInput").ap()
    w = nc.dram_tensor("w", (D_IN, D_OUT), dt_w, kind="ExternalInput").ap()
    b = nc.dram_tensor("b", (1, D_OUT), f32, kind="ExternalInput").ap()
    y = nc.dram_tensor("y", (CAP, D_OUT), dt_y, kind="ExternalOutput").ap()

    with tile.TileContext(nc) as tc:
        with (
            tc.tile_pool(name="wpool", bufs=1) as wpool,
            tc.tile_pool(name="bias", bufs=1) as bias_pool,
            tc.tile_pool(name="xslab", bufs=3) as xpool,
            tc.tile_pool(name="ystage", bufs=3) as ypool,
            tc.tile_pool(name="psum", bufs=6, space="PSUM") as pspool,
            tc.tile_pool(name="psum_b", bufs=1, space="PSUM") as psb_pool,
        ):
            # Slab schedule: small first slab so matmuls start early, small
            # last slab so the tail flush (DVE + out-DMA after last MM) is
            # short.
            sizes = [256]
            remaining = CAP - 256 - 256
            while remaining > 0:
                sizes.append(min(TOK_SLAB, remaining))
                remaining -= sizes[-1]
            sizes.append(256)
            slabs = []
            t0 = 0
            for ts in sizes:
                slabs.append((t0, ts))
                t0 += ts
            assert t0 == CAP

            # Weights: separate tile per k-chunk so the first matmuls only
            # gate on chunk 0 (256KB) instead of the full 1MB.
            w_sbs = [
                wpool.tile([128, D_OUT], dt_w, tag=f"w{k}", name=f"w_sb{k}")
                for k in range(KC)
            ]

            def load_w(k):
                nc.sync.dma_start(w_sbs[k][:], w[k * 128 : (k + 1) * 128, :])

            def load_x(slab_i):
                t0, ts = slabs[slab_i]
                xs = xpool.tile([128, KC * ts], dt_x, tag="xs")
                nc.sync.dma_start(
                    xs[:].rearrange("p (kc t) -> p kc t", kc=KC),
                    xt[:, t0 : t0 + ts].rearrange("(kc p) t -> p kc t", p=128),
                )
                return xs

            # HWDGE queue order: bias (2KB, feeds the first PE op), w0,
            # x-slab0, w1..w3 — the first GEMM matmul gates on w0+slab0
            # only; remaining W chunks stream behind.
            b_sb1 = bias_pool.tile([1, D_OUT], f32, tag="b1")
            nc.sync.dma_start(b_sb1[:], b[:])
            load_w(0)
            xs_pending = load_x(0)
            for k in range(1, KC):
                load_w(k)

            # Bias: replicate across 128 partitions via ones-matmul
            # (lhsT = ones[1,128], rhs = b[1,512]).
            ones = bias_pool.tile([1, 128], f32, tag="ones")
            nc.any.memset(ones[:], 1.0)
            b_ps = psb_pool.tile([128, D_OUT], f32, tag="bps")
            nc.tensor.matmul(b_ps[:], ones[:], b_sb1[:], start=True, stop=True)
            b_rep = bias_pool.tile([128, D_OUT], f32, tag="brep")
            nc.vector.tensor_copy(b_rep[:], b_ps[:])

            for i, (t0, ts) in enumerate(slabs):
                nt = ts // 128
                xs = xs_pending
                if i + 1 < len(slabs):
                    xs_pending = load_x(i + 1)
                ys = ypool.tile([128, nt * D_OUT], dt_y, tag="ys")
                for a in range(nt):
                    ps = pspool.tile([128, D_OUT], f32, tag="acc")
                    for k in range(KC):
                        nc.tensor.matmul(
                            ps[:],
                            xs[:, k * ts + a * 128 : k * ts + (a + 1) * 128],
                            w_sbs[k][:],
                            start=(k == 0),
                            stop=(k == KC - 1),
                        )
                    nc.vector.tensor_add(
                        ys[:, a * D_OUT : (a + 1) * D_OUT], ps[:], b_rep[:]
                    )
                # Output on the ACT HWDGE ring — separate FIFO from inputs.
                nc.scalar.dma_start(
                    y[t0 : t0 + ts, :].rearrange("(a p) o -> p a o", p=128),
                    ys[:].rearrange("p (a o) -> p a o", a=nt),
                )
    nc.compile()
    return nc


def _get_nc(mm_dtype_name):
    if mm_dtype_name not in _cache:
        _cache[mm_dtype_name] = _build(mm_dtype_name)
    return _cache[mm_dtype_name]


def kernel(x, index, weight, bias, _trace=False):
    from concourse.bass_utils import run_bass_kernel_spmd

    x = np.ascontiguousarray(np.asarray(x, dtype=np.float32))
    weight = np.ascontiguousarray(np.asarray(weight, dtype=np.float32))
    bias = np.ascontiguousarray(np.asarray(bias, dtype=np.float32))
    idx = np.asarray(index).astype(np.int64, copy=False)

    ids = [np.nonzero(idx == e)[0] for e in range(N_EXPERTS)]

    in_maps = []
    for e in range(N_EXPERTS):
        n_e = min(len(ids[e]), CAP)
        xt_e = np.zeros((D_IN, CAP), dtype=np.float32)
        xt_e[:, :n_e] = x[ids[e][:n_e]].T
        in_maps.append(
            {
                "xt": xt_e,
                "w": weight[e],
                "b": bias[e : e + 1],
            }
        )

    x_dt_name, w_dt_name, y_dt_name = _DT_MAP[MM_DTYPE]
    _np_dt = {"bfloat16": None, "float16": np.float16}
    if x_dt_name in _np_dt or w_dt_name in _np_dt:
        import ml_dtypes

        cast = {
            "bfloat16": ml_dtypes.bfloat16,
            "float16": np.float16,
        }
        if x_dt_name in cast:
            in_maps = [
                {**m, "xt": m["xt"].astype(cast[x_dt_name])} for m in in_maps
            ]
        if w_dt_name in cast:
            in_maps = [
                {**m, "w": m["w"].astype(cast[w_dt_name])} for m in in_maps
            ]

    nc = _get_nc(MM_DTYPE)
    res = run_bass_kernel_spmd(
        nc, in_maps, core_ids=list(range(N_EXPERTS)), trace=_trace
    )

    out = np.empty((x.shape[0], D_OUT), dtype=np.float32)
    for e in range(N_EXPERTS):
        n_e = min(len(ids[e]), CAP)
        out[ids[e][:n_e]] = res.results[e]["y"][:n_e].astype(np.float32)
        if len(ids[e]) > CAP:  # capacity overflow: host fallback (correctness net)
            over = ids[e][CAP:]
            out[over] = x[over] @ weight[e] + bias[e]

    if _trace:
        return out, res
    return out


# revision 14
# speedup vs baseline: 1.0226x; 1.0226x over previous
"""MoE expert-routing kernel for Trainium2 (8 NeuronCores, expert-parallel).

Problem: out[t] = x[t] @ weight[index[t]] + bias[index[t]]
  x: (32768, 512) f32, index: (32768,) int, weight: (8, 512, 512) f32,
  bias: (8, 512) f32.

Strategy (expert-parallel, host-side dispatch):
  Core e owns expert e. The host gathers the tokens routed to expert e
  into a fixed-capacity, transposed buffer xt_e[512, CAP] (padded with
  zeros), and core e computes y_e = x_e @ W_e + b_e as a single dense
  GEMM. Results are scattered back to token order on the host. Tokens
  beyond CAP (never happens for the benchmark distribution: binomial
  max ~4205 << CAP) fall back to a host matmul for correctness.

Device kernel (per core): y[CAP, 512] = xt.T @ w + b
  - xt arrives pre-transposed [512(k), CAP(t)] so the K-contraction dim
    lands on SBUF partitions with no on-device transposes.
  - Token slabs of 512 stream through SBUF; per 128-token tile, 4
    accumulating matmuls (K=128 chunks) into one PSUM bank; DVE adds the
    (pre-replicated) bias while moving PSUM->SBUF; slab-sized output DMA.
"""

import os

import numpy as np

N_EXPERTS = 8
D_IN = 512
D_OUT = 512
N_TOKENS = 32768
CAP = 4352  # per-expert token capacity: 34*128, binomial(32768,1/8) max +4sigma
TOK_SLAB = 256
KC = D_IN // 128  # 4 contraction chunks

# Matmul operand dtypes (x, w):
#   "float32"  = full fp32 (4 cyc/row on PE, ~1.6e-7 rel err)
#   "float32r" = PE fast-fp32 (tf32-like, ~1.4e-4 rel err)
#   "bf16x"    = x in bf16, w in float32r (~? rel err, halves x DMA)
#   "bfloat16" = both bf16 (~2e-3 rel err, min DMA)
MM_DTYPE = os.environ.get("KERNEL_MM_DTYPE", "float32r_o16")
# mode -> (x dtype, w dtype, y dtype)
_DT_MAP = {
    "float32": ("float32", "float32", "float32"),
    "float32r": ("float32r", "float32r", "float32"),
    "float32r_o16": ("float32r", "float32r", "float16"),
    "bf16x": ("bfloat16", "float32r", "float32"),
    "bfloat16": ("bfloat16", "bfloat16", "float32"),
    "float16": ("float16", "float16", "float32"),
    "float16_o16": ("float16", "float16", "float16"),
}

_cache = {}


def _build(mm_dtype_name):
    import concourse.bacc as bacc
    import concourse.mybir as mybir
    import concourse.tile as tile

    x_dt_name, w_dt_name, y_dt_name = _DT_MAP[mm_dtype_name]
    dt_x = getattr(mybir.dt, x_dt_name)
    dt_w = getattr(mybir.dt, w_dt_name)
    dt_y = getattr(mybir.dt, y_dt_name)
    f32 = mybir.dt.float32

    nc = bacc.Bacc("TRN2", target_bir_lowering=False, debug=False, num_devices=N_EXPERTS)
    xt = nc.dram_tensor("xt", (D_IN, CAP), dt_x, kind="ExternalInput").ap()
    w = nc.dram_tensor("w", (D_IN, D_OUT), dt_w, kind="ExternalInput").ap()
    b = nc.dram_tensor("b", (1, D_OUT), f32, kind="ExternalInput").ap()
    y = nc.dram_tensor("y", (CAP, D_OUT), dt_y, kind="ExternalOutput").ap()

    with tile.TileContext(nc) as tc:
        with (
            tc.tile_pool(name="wpool", bufs=1) as wpool,
            tc.tile_pool(name="bias", bufs=1) as bias_pool,
            tc.tile_pool(name="xslab", bufs=8) as xpool,
            tc.tile_pool(name="ystage", bufs=8) as ypool,
            tc.tile_pool(name="psum", bufs=6, space="PSUM") as pspool,
            tc.tile_pool(name="psum_b", bufs=1, space="PSUM") as psb_pool,
        ):
            # Slab schedule: small first slab so matmuls start early, small
            # last slab so the tail flush (DVE + out-DMA after last MM) is
            # short.
            sizes = [256]
            remaining = CAP - 256 - 256
            while remaining > 0:
                sizes.append(min(TOK_SLAB, remaining))
                remaining -= sizes[-1]
            sizes.append(256)
            slabs = []
            t0 = 0
            for ts in sizes:
                slabs.append((t0, ts))
                t0 += ts
            assert t0 == CAP

            # Weights: separate tile per k-chunk so the first matmuls only
            # gate on chunk 0 (256KB) instead of the full 1MB.
            w_sbs = [
                wpool.tile([128, D_OUT], dt_w, tag=f"w{k}", name=f"w_sb{k}")
                for k in range(KC)
            ]

            def load_w(k):
                nc.sync.dma_start(w_sbs[k][:], w[k * 128 : (k + 1) * 128, :])

            def load_x(slab_i):
                t0, ts = slabs[slab_i]
                xs = xpool.tile([128, KC * ts], dt_x, tag="xs")
                nc.sync.dma_start(
                    xs[:].rearrange("p (kc t) -> p kc t", kc=KC),
                    xt[:, t0 : t0 + ts].rearrange("(kc p) t -> p kc t", p=128),
                )
                return xs

            # HWDGE queue order: bias (2KB, feeds the first PE op), w0,
            # x-slab0, w1..w3 — the first GEMM matmul gates on w0+slab0
            # only; remaining W chunks stream behind.
            b_sb1 = bias_pool.tile([1, D_OUT], f32, tag="b1")
            nc.sync.dma_start(b_sb1[:], b[:])
            load_w(0)
            xs_pending = load_x(0)
            for k in range(1, KC):
                load_w(k)

            # Bias: replicate across 128 partitions via ones-matmul
            # (lhsT = ones[1,128], rhs = b[1,512]).
            ones = bias_pool.tile([1, 128], f32, tag="ones")
            nc.any.memset(ones[:], 1.0)
            b_ps = psb_pool.tile([128, D_OUT], f32, tag="bps")
            nc.tensor.matmul(b_ps[:], ones[:], b_sb1[:], start=True, stop=True)
            b_rep = bias_pool.tile([128, D_OUT], f32, tag="brep")
            nc.vector.tensor_copy(b_rep[:], b_ps[:])

            for i, (t0, ts) in enumerate(slabs):
                nt = ts // 128
                xs = xs_pending
                if i + 1 < len(slabs):
                    xs_pending = load_x(i + 1)
                ys = ypool.tile([128, nt * D_OUT], dt_y, tag="ys")
                for a in range(nt):
                    ps = pspool.tile([128, D_OUT], f32, tag="acc")
                    for k in range(KC):
                        nc.tensor.matmul(
                            ps[:],
                            xs[:, k * ts + a * 128 : k * ts + (a + 1) * 128],
                            w_sbs[k][:],
                            start=(k == 0),
                            stop=(k == KC - 1),
                        )
                    nc.vector.tensor_add(
                        ys[:, a * D_OUT : (a + 1) * D_OUT], ps[:], b_rep[:]
                    )
                # Output on the ACT HWDGE ring — separate FIFO from inputs.
                nc.scalar.dma_start(
                    y[t0 : t0 + ts, :].rearrange("(a p) o -> p a o", p=128),
                    ys[:].rearrange("p (a o) -> p a o", a=nt),
                )
    nc.compile()
    return nc


def _get_nc(mm_dtype_name):
    if mm_dtype_name not in _cache:
        _cache[mm_dtype_name] = _build(mm_dtype_name)
    return _cache[mm_dtype_name]


def kernel(x, index, weight, bias, _trace=False):
    from concourse.bass_utils import run_bass_kernel_spmd

    x = np.ascontiguousarray(np.asarray(x, dtype=np.float32))
    weight = np.ascontiguousarray(np.asarray(weight, dtype=np.float32))
    bias = np.ascontiguousarray(np.asarray(bias, dtype=np.float32))
    idx = np.asarray(index).astype(np.int64, copy=False)

    ids = [np.nonzero(idx == e)[0] for e in range(N_EXPERTS)]

    in_maps = []
    for e in range(N_EXPERTS):
        n_e = min(len(ids[e]), CAP)
        xt_e = np.zeros((D_IN, CAP), dtype=np.float32)
        xt_e[:, :n_e] = x[ids[e][:n_e]].T
        in_maps.append(
            {
                "xt": xt_e,
                "w": weight[e],
                "b": bias[e : e + 1],
            }
        )

    x_dt_name, w_dt_name, y_dt_name = _DT_MAP[MM_DTYPE]
    _np_dt = {"bfloat16": None, "float16": np.float16}
    if x_dt_name in _np_dt or w_dt_name in _np_dt:
        import ml_dtypes

        cast = {
            "bfloat16": ml_dtypes.bfloat16,
            "float16": np.float16,
        }
        if x_dt_name in cast:
            in_maps = [
                {**m, "xt": m["xt"].astype(cast[x_dt_name])} for m in in_maps
            ]
        if w_dt_name in cast:
            in_maps = [
                {**m, "w": m["w"].astype(cast[w_dt_name])} for m in in_maps
            ]

    nc = _get_nc(MM_DTYPE)
    res = run_bass_kernel_spmd(
        nc, in_maps, core_ids=list(range(N_EXPERTS)), trace=_trace
    )

    out = np.empty((x.shape[0], D_OUT), dtype=np.float32)
    for e in range(N_EXPERTS):
        n_e = min(len(ids[e]), CAP)
        out[ids[e][:n_e]] = res.results[e]["y"][:n_e].astype(np.float32)
        if len(ids[e]) > CAP:  # capacity overflow: host fallback (correctness net)
            over = ids[e][CAP:]
            out[over] = x[over] @ weight[e] + bias[e]

    if _trace:
        return out, res
    return out
